# revision 1
# baseline (speedup 1.0000x reference)
"""MoE conv layer (top-2 of 4 experts, 3x3 SAME conv + FiLM) on 8 TRN2 cores.

Strategy: data-parallel over batch (2 images per core). The gate
(global-avg-pool -> linear -> softmax -> top-2) is tiny and runs on the
host; each core receives only its 2 images plus the selected 2 experts'
conv weights per image, pre-transposed/stacked for the PE array, with the
FiLM scale k[b,c] and the softmax weight folded into the conv weights
(valid since both are >= 0 and relu(z)*s == relu(z*s) for s >= 0).

Device kernel, per image, per 32-row group:
  One full-width DMA loads x2 [128, WIN]: 64 channels of the zero-padded
  image window in one partition half and the same window shifted by +2
  columns in the other half (the DRAM window is read twice -- this keeps
  all 16 SBUF ports busy instead of 8).
  conv as 6 matmuls per 512-pixel (4-row) tile accumulating in PSUM:
    - 3 "pair" matmuls (K=128): taps (ty,0) and (ty,2) together via the
      shifted stacking; both experts packed in M=128 (e0 -> psum
      partitions 0-63, e1 -> 64-127).
    - 3 "single" matmuls (K=64): taps (ty,1) from the unshifted half.
  epilogue per tile:
    ACT: r = relu(psum + bias)            (bias = conv_b * k * w)
    PE : ps2 = [I;I].T @ r                (cross-partition expert sum)
    DVE: out2[half] = ps2 + x             (residual, lane-aligned)
  Odd groups mirror the partition roles (x in the upper half, epilogue in
  lanes 64-127) so that a group PAIR fills out2 [128, 4096+4096], which is
  stored with one full-width DMA.
"""

import contextlib

import numpy as np

import concourse.bass as bass
import concourse.tile as tile
from concourse import bacc, mybir
from concourse.ap import AP
from concourse.bass_utils import run_bass_kernel_spmd

N_CORES = 8
B, C, H, W = 16, 64, 128, 128
BPC = B // N_CORES            # images per core
HP, WP = H + 2, W + 2         # zero-padded image dims (SAME conv)
GROUP_ROWS = 32               # output rows per x-window group
N_GROUPS = H // GROUP_ROWS    # 4 (processed as 2 pairs)
TILE_ROWS = 4                 # output rows per matmul tile (N = 4*128 = 512)
TILES_PER_GROUP = GROUP_ROWS // TILE_ROWS
NT = TILE_ROWS * W            # matmul free size (512)
WIN = (GROUP_ROWS + 2) * WP   # x-window cols per group (incl. 2 halo rows)
XW = WIN + 2                  # x2 tile width (+2 so odd-group reads stay in range)
GPIX = GROUP_ROWS * W         # output pixels per group (4096)
XPITCH = HP * WP + 6          # per-channel pitch in DRAM xpad (2 lead, 4 trail pad)
XS = 2                        # offset of the real window start within a channel

F32 = mybir.dt.float32
F32R = mybir.dt.float32r

_NC_CACHE = None


def _rows_ap(t_ap, base_col, p0, p1, col0, ncol=W):
    """TILE_ROWS rows of stride WP starting at base_col within an x2 tile."""
    v = t_ap[p0:p1, base_col:base_col + TILE_ROWS * WP]
    v = v.rearrange("p (r c) -> p r c", c=WP)
    return v[:, :, col0:col0 + ncol]


def _build_nc(loop_n=None):
    nc = bacc.Bacc("TRN2", target_bir_lowering=False, debug=False)
    xpad = nc.dram_tensor("xpad", [BPC, C, XPITCH], F32R, kind="ExternalInput").ap()
    wpr = nc.dram_tensor("wpr", [BPC, 3, 2 * C, 2 * C], F32R, kind="ExternalInput").ap()
    wsg = nc.dram_tensor("wsg", [BPC, 3, 2 * C, 2 * C], F32R, kind="ExternalInput").ap()
    bias = nc.dram_tensor("bias", [2 * C, BPC], F32, kind="ExternalInput").ap()
    sel = nc.dram_tensor("sel", [2 * C, C], F32R, kind="ExternalInput").ap()
    out = nc.dram_tensor("out", [BPC, C, H * W], F32, kind="ExternalOutput").ap()

    with tile.TileContext(nc) as tc:
        loop_ctx = (
            tc.For_i(0, loop_n, 1, hint_engines=(
                mybir.EngineType.PE, mybir.EngineType.Activation,
                mybir.EngineType.DVE, mybir.EngineType.SP,
                mybir.EngineType.Pool,
            ))
            if loop_n is not None else contextlib.nullcontext()
        )
        with (
            loop_ctx,
            tc.tile_pool(name="const", bufs=1) as cpool,
            tc.tile_pool(name="wpool", bufs=6) as wpool,
            tc.tile_pool(name="xpool", bufs=4) as xpool,
            tc.tile_pool(name="rpool", bufs=6) as rpool,
            tc.tile_pool(name="opool", bufs=2) as opool,
            tc.tile_pool(name="psA", bufs=4, space="PSUM") as psA,
            tc.tile_pool(name="psB", bufs=3, space="PSUM") as psB,
        ):
            sel_t = cpool.tile([2 * C, C], F32R)
            nc.sync.dma_start(out=sel_t, in_=sel)
            bias_t = cpool.tile([2 * C, BPC], F32)
            nc.sync.dma_start(out=bias_t, in_=bias)
            for b in range(BPC):
                wp_t, ws_t = [], []
                for ty in range(3):
                    wt = wpool.tile([2 * C, 2 * C], F32R, tag="wp")
                    nc.sync.dma_start(out=wt, in_=wpr[b, ty])
                    wp_t.append(wt)
                    # K=128 with zero upper-half weights: K<128 f32r matmuls
                    # run at ~1/4 rate, so pad K instead
                    st = wpool.tile([2 * C, 2 * C], F32R, tag="ws")
                    nc.sync.dma_start(out=st, in_=wsg[b, ty])
                    ws_t.append(st)
                for p in range(N_GROUPS // 2):
                    out2 = opool.tile([C, 2 * GPIX], F32, tag="o2")
                    for j in range(2):
                        g = 2 * p + j
                        row0 = GROUP_ROWS * g
                        x2 = xpool.tile([2 * C, XW], F32R, tag="x2")
                        # even groups: lo = x, hi = x shifted +2 (read at o:
                        #   lo = x[o], hi = x[o+2])
                        # odd groups:  lo = x shifted -2, hi = x (read at o:
                        #   lo = x[o-2], hi = x[o])
                        base = b * C * XPITCH + XS + WP * row0 - 2 * j
                        src_lo = AP(xpad.tensor, base, [[XPITCH, C], [1, XW]])
                        src_hi = AP(xpad.tensor, base + 2, [[XPITCH, C], [1, XW]])
                        nc.sync.dma_start(out=x2[0:C, :], in_=src_lo)
                        nc.sync.dma_start(out=x2[C:2 * C, :], in_=src_hi)
                        # all epilogue reads use the UNSHIFTED copy: the lower
                        # half holds x shifted by -2j, so shift read cols by +2j
                        co = 2 * j
                        for th in range(TILES_PER_GROUP // 4):
                            pss = []
                            for q in range(4):
                                psq = psA.tile([2 * C, NT], F32, tag="psA",
                                               name=f"ps_{b}_{g}_{th}_{q}")
                                pss.append(psq)
                            # weight-stationary: each lhsT feeds 4 consecutive
                            # matmuls (fused f32r matmuls reload weights per
                            # instruction; consecutive reuse is cheaper)
                            for ty in range(3):
                                for q in range(4):
                                    lr = TILE_ROWS * (4 * th + q)
                                    rp = _rows_ap(x2, (lr + ty) * WP, 0, 2 * C, co)
                                    nc.tensor.matmul(
                                        pss[q], wp_t[ty], rp,
                                        start=(ty == 0), stop=False,
                                    )
                                for q in range(4):
                                    lr = TILE_ROWS * (4 * th + q)
                                    rs = _rows_ap(x2, (lr + ty) * WP + co, 0, 2 * C, 1)
                                    nc.tensor.matmul(
                                        pss[q], ws_t[ty], rs,
                                        start=False, stop=(ty == 2),
                                    )
                            for q in range(4):
                                t = 4 * th + q
                                lr = TILE_ROWS * t
                                ps = pss[q]
                                r = rpool.tile([2 * C, NT], F32R, tag="r",
                                               name=f"r_{b}_{g}_{th}_{q}")
                                nc.scalar.activation(
                                    out=r, in_=ps,
                                    func=mybir.ActivationFunctionType.Relu,
                                    bias=bias_t[:, b:b + 1], scale=1.0,
                                )
                                ps2 = psB.tile([C, NT], F32, tag="psB",
                                               name=f"ps2_{b}_{g}_{th}_{q}")
                                nc.tensor.matmul(ps2, sel_t, r, start=True, stop=True)
                                xres = _rows_ap(x2, (lr + 1) * WP + co, 0, C, 1)
                                nc.vector.tensor_add(
                                out=out2[:, GPIX * j + NT * t:GPIX * j + NT * (t + 1)],
                                in0=ps2, in1=xres.bitcast(F32),
                            )
                    # store the group pair (contiguous per channel in DRAM)
                    obase = b * C * H * W + 2 * p * GPIX
                    dsto = AP(out.tensor, obase, [[H * W, C], [1, 2 * GPIX]])
                    nc.sync.dma_start(out=dsto, in_=out2)
    nc.compile()
    return nc


def _get_nc():
    global _NC_CACHE
    if _NC_CACHE is None:
        _NC_CACHE = _build_nc()
    return _NC_CACHE


def _route(x, k, gate_w, gate_b):
    """Host gate: returns (idx [B,2] int, topw [B,2] f32), matching
    softmax(fp32) + lax.top_k (descending, ties -> lower index)."""
    pooled = x.mean(axis=(2, 3), dtype=np.float32)
    logits = (pooled @ gate_w.T + gate_b).astype(np.float32)
    m = logits.max(axis=1, keepdims=True)
    e = np.exp(logits - m)
    wts = (e / e.sum(axis=1, keepdims=True)).astype(np.float32)
    idx = np.argsort(-wts, axis=1, kind="stable")[:, :2]
    topw = np.take_along_axis(wts, idx, axis=1)
    return idx, topw


def _prep_inputs(inputs, k, gate_w, gate_b, conv_w, conv_b):
    x = np.asarray(inputs, np.float32)
    k = np.asarray(k, np.float32)
    gate_w = np.asarray(gate_w, np.float32)
    gate_b = np.asarray(gate_b, np.float32)
    conv_w = np.asarray(conv_w, np.float32)
    conv_b = np.asarray(conv_b, np.float32)

    idx, topw = _route(x, k, gate_w, gate_b)
    kk = k[:, :, 0, 0]                                  # [B, C]
    s = kk[:, None, :] * topw[:, :, None]               # [B, 2, C_out]
    wsel = conv_w[idx]                                  # [B, 2, co, ci, 3, 3]
    wsc = wsel * s[:, :, :, None, None, None]
    lhsT = np.transpose(wsc, (0, 4, 5, 3, 1, 2)).reshape(B, 3, 3, C, 2 * C)
    # pair slots: rows 0-63 = tap (ty,0), rows 64-127 = tap (ty,2)
    wpr = np.stack(
        [np.concatenate([lhsT[:, ty, 0], lhsT[:, ty, 2]], axis=1) for ty in range(3)],
        axis=1,
    )                                                   # [B, 3, 128, 128]
    wsg = np.zeros((B, 3, 2 * C, 2 * C), np.float32)    # K padded with zeros
    wsg[:, :, :C, :] = lhsT[:, :, 1]
    bias_full = (conv_b[idx] * s).reshape(B, 2 * C)     # [B, 128]
    xpad = np.zeros((B, C, XPITCH), np.float32)
    pad2d = np.zeros((B, C, HP, WP), np.float32)
    pad2d[:, :, 1:H + 1, 1:W + 1] = x
    xpad[:, :, XS:XS + HP * WP] = pad2d.reshape(B, C, HP * WP)
    sel_in = np.ascontiguousarray(
        np.concatenate([np.eye(C, dtype=np.float32)] * 2, axis=0)
    )

    in_maps = []
    for c in range(N_CORES):
        sl = slice(c * BPC, (c + 1) * BPC)
        in_maps.append({
            "xpad": np.ascontiguousarray(xpad[sl]),
            "wpr": np.ascontiguousarray(wpr[sl]),
            "wsg": np.ascontiguousarray(wsg[sl]),
            "bias": np.ascontiguousarray(bias_full[sl].T),
            "sel": sel_in,
        })
    return in_maps


def _run(inputs, k, gate_w, gate_b, conv_w, conv_b, **run_kwargs):
    in_maps = _prep_inputs(inputs, k, gate_w, gate_b, conv_w, conv_b)
    nc = _get_nc()
    res = run_bass_kernel_spmd(nc, in_maps, core_ids=list(range(N_CORES)), **run_kwargs)
    outs = [res.results[c]["out"].reshape(BPC, C, H, W) for c in range(N_CORES)]
    full = np.concatenate(outs, axis=0).astype(np.float32)
    return full, res


def kernel(inputs, k, gate_w, gate_b, conv_w, conv_b):
    full, _ = _run(inputs, k, gate_w, gate_b, conv_w, conv_b)
    return full



# revision 2
# speedup vs baseline: 1.1899x; 1.1899x over previous
"""MoE conv layer (top-2 of 4 experts, 3x3 SAME conv + FiLM) on 8 TRN2 cores.

Strategy: data-parallel over batch (2 images per core); the tiny gate
(global-avg-pool -> linear -> softmax -> top-2) runs on the host and the
FiLM scale k and softmax weight are folded into the conv weights/bias
(valid since both are >= 0 and relu(z)*s == relu(z*s) for s >= 0).
bf16 datapath (psum f32), software-pipelined epilogue, staggered-reset
timing loop.
Stores + const loads ride the ACT engine HWDGE queue (SP keeps x/w loads);
first window split into quarters so the first matmul starts ~2us in;
each 4-tile block stores its half-group as soon as its epilogue runs.

Data-parallel over batch (2 images per core). Host gate folds k*softmax
weight into conv weights (relu(z*s) == relu(z)*s for s >= 0).

Device kernel, per image, per 32-row group (all tensors bf16, psum f32):
  Two half-window DMAs load x2 [128, XW]: 64 channels of the zero-padded
  window (lo) and the same window shifted +2 cols (hi) — two DMAs, not one
  fused, and K=128 everywhere: measured faster on HW than the "obvious"
  single-DMA / K=64 variants.
  conv per 512-pixel (4-row) tile: 6 matmuls accumulating in PSUM
    - 3 "pair" matmuls (K=128): taps (ty,0)+(ty,2) via the shifted stacking
    - 3 "single" matmuls (K=128, hi half zero weights): taps (ty,1)
    both experts packed in M=128 (e0 -> psum 0-63, e1 -> 64-127)
  epilogue per tile (pipelined one 4-tile block behind the convs so PE
  never waits on ACT):
    ACT: r = relu(psum + bias)            (bias = conv_b * k * w)
    PE : psum[0:64] = [I;I].T @ r         (expert fold, written back into
                                           the SAME psum tile - no extra
                                           PSUM banks, psA uses all 8)
    DVE: og[tile] = psum[0:64] + x        (residual, lane-aligned)
  one store DMA per group (bf16; host casts back to f32).
"""

import contextlib

import numpy as np
import ml_dtypes

import concourse.bass as bass
import concourse.tile as tile
from concourse import bacc, mybir
from concourse.ap import AP
from concourse.bass_utils import run_bass_kernel_spmd

N_CORES = 8
B, C, H, W = 16, 64, 128, 128
BPC = B // N_CORES            # images per core
HP, WP = H + 2, W + 2         # zero-padded image dims (SAME conv)
GROUP_ROWS = 32               # output rows per x-window group
N_GROUPS = H // GROUP_ROWS    # 4
TILE_ROWS = 4                 # output rows per matmul tile (N = 4*128 = 512)
TILES_PER_GROUP = GROUP_ROWS // TILE_ROWS   # 8
NT = TILE_ROWS * W            # matmul free size (512)
WIN = (GROUP_ROWS + 2) * WP   # x-window cols per group (incl. 2 halo rows)
XW = WIN + 2                  # x2 tile width (+2 so odd-group reads stay in range)
GPIX = GROUP_ROWS * W         # output pixels per group (4096)
XPITCH = HP * WP + 2 + WP + 4  # per-channel pitch (2 lead, WP+4 trail for xB hi)
XS = 2                        # offset of the real window start within a channel

F32 = mybir.dt.float32
BF16 = mybir.dt.bfloat16

_NC_CACHE = None


def _rows_ap(t_ap, base_col, p0, p1, col0, ncol=W):
    """TILE_ROWS rows of stride WP starting at base_col within an x2 tile."""
    v = t_ap[p0:p1, base_col:base_col + TILE_ROWS * WP]
    v = v.rearrange("p (r c) -> p r c", c=WP)
    return v[:, :, col0:col0 + ncol]


def _build_nc(loop_n=None):
    nc = bacc.Bacc("TRN2", target_bir_lowering=False, debug=False)
    xpad = nc.dram_tensor("xpad", [BPC, C, XPITCH], BF16, kind="ExternalInput").ap()
    wpr = nc.dram_tensor("wpr", [BPC, 3, 2 * C, 2 * C], BF16, kind="ExternalInput").ap()
    wmid = nc.dram_tensor("wmid", [BPC, 2, 2 * C, 2 * C], BF16, kind="ExternalInput").ap()
    bias = nc.dram_tensor("bias", [2 * C, BPC], F32, kind="ExternalInput").ap()
    sel = nc.dram_tensor("sel", [2 * C, C], BF16, kind="ExternalInput").ap()
    out = nc.dram_tensor("out", [BPC, C, H * W], BF16, kind="ExternalOutput").ap()

    with tile.TileContext(nc) as tc:
        loop_ctx = (
            tc.For_i(0, loop_n, 1, staggered_reset=True, hint_engines=(
                mybir.EngineType.PE, mybir.EngineType.Activation,
                mybir.EngineType.DVE, mybir.EngineType.SP,
            ))
            if loop_n is not None else contextlib.nullcontext()
        )
        with (
            loop_ctx,
            tc.tile_pool(name="const", bufs=1) as cpool,
            tc.tile_pool(name="wpool", bufs=6) as wpool,
            tc.tile_pool(name="xpool", bufs=5) as xpool,
            tc.tile_pool(name="rpool", bufs=6) as rpool,
            tc.tile_pool(name="opool", bufs=3) as opool,
            tc.tile_pool(name="psA", bufs=8, space="PSUM") as psA,
        ):
            # first window of the first image before anything else, split in
            # row-halves: the first 4-tile block only needs rows < 18, so the
            # first matmul can start after the first quarter lands
            x2_first = xpool.tile([2 * C, XW], BF16, tag="x2")
            base0 = 0 * C * XPITCH + XS
            XSPL = 18 * WP
            for (p0, p1, off) in ((0, C, 0), (C, 2 * C, 2)):
                nc.sync.dma_start(
                    out=x2_first[p0:p1, 0:XSPL],
                    in_=AP(xpad.tensor, base0 + off, [[XPITCH, C], [1, XSPL]]))
            for (p0, p1, off) in ((0, C, 0), (C, 2 * C, 2)):
                nc.sync.dma_start(
                    out=x2_first[p0:p1, XSPL:XW],
                    in_=AP(xpad.tensor, base0 + off + XSPL,
                           [[XPITCH, C], [1, XW - XSPL]]))
            # consts off the SP queue: ACT hosts its own HWDGE
            sel_t = cpool.tile([2 * C, C], BF16)
            nc.scalar.dma_start(out=sel_t, in_=sel)
            bias_t = cpool.tile([2 * C, BPC], F32)
            nc.scalar.dma_start(out=bias_t, in_=bias)

            # pending epilogue from the previous 4-tile block:
            # (pss, rts, og, x2, co, th, store) -- emitted AFTER the next
            # block's conv matmuls so PE never waits on ACT; the group store
            # rides along with the group's last block so it lands after the
            # DVE writes it needs
            pend = []

            def flush_pend():
                for (pss, rts, og, x2v, co, th, store) in pend:
                    for q in range(4):
                        t = 4 * th + q
                        lr = TILE_ROWS * t
                        ps2 = pss[q][0:C, :]
                        nc.tensor.matmul(ps2, sel_t, rts[q], start=True,
                                         stop=True)
                        xres = _rows_ap(x2v, (lr + 1) * WP + co, 0, C, 1)
                        nc.vector.tensor_add(
                            out=og[:, NT * t:NT * (t + 1)],
                            in0=ps2, in1=xres,
                        )
                    if store is not None:
                        nc.scalar.dma_start(out=store, in_=og[:, NT * 4 * th:
                                                              NT * 4 * (th + 1)])
                pend.clear()

            for b in range(BPC):
                wp_t, wm_t = [], []
                for ty in range(3):
                    wt = wpool.tile([2 * C, 2 * C], BF16, tag="wp")
                    nc.sync.dma_start(out=wt, in_=wpr[b, ty])
                    wp_t.append(wt)
                for mi in range(2):
                    # wm[0]: K lo=tap(0,1), hi=tap(1,1) pair via xB
                    # wm[1]: K lo=tap(2,1), hi=zeros (K<128 measures slower
                    # on HW, so pad K)
                    st = wpool.tile([2 * C, 2 * C], BF16, tag="ws")
                    nc.sync.dma_start(out=st, in_=wmid[b, mi])
                    wm_t.append(st)
                for g in range(N_GROUPS):
                    j = g % 2
                    row0 = GROUP_ROWS * g
                    if b == 0 and g == 0:
                        x2 = x2_first
                    else:
                        x2 = xpool.tile([2 * C, XW], BF16, tag="x2")
                        # even groups: lo = x, hi = x shifted +2
                        # odd groups:  lo = x shifted -2, hi = x
                        base = b * C * XPITCH + XS + WP * row0 - 2 * j
                        src_lo = AP(xpad.tensor, base, [[XPITCH, C], [1, XW]])
                        src_hi = AP(xpad.tensor, base + 2, [[XPITCH, C], [1, XW]])
                        nc.sync.dma_start(out=x2[0:C, :], in_=src_lo)
                        nc.sync.dma_start(out=x2[C:2 * C, :], in_=src_hi)
                    xB = xpool.tile([2 * C, XW], BF16, tag="xb", bufs=4)
                    # lo = x (shifted -2j), hi = x shifted +WP (one row down)
                    baseB = b * C * XPITCH + XS + WP * row0 - 2 * j
                    nc.sync.dma_start(
                        out=xB[0:C, :],
                        in_=AP(xpad.tensor, baseB, [[XPITCH, C], [1, XW]]))
                    nc.sync.dma_start(
                        out=xB[C:2 * C, :],
                        in_=AP(xpad.tensor, baseB + WP, [[XPITCH, C], [1, XW]]))
                    # epilogue reads use the UNSHIFTED copy: shift cols by +2j
                    co = 2 * j
                    og = opool.tile([C, GPIX], BF16, tag="og")
                    for th in range(TILES_PER_GROUP // 4):
                        pss = []
                        for q in range(4):
                            psq = psA.tile([2 * C, NT], F32, tag="psA",
                                           name=f"ps_{b}_{g}_{th}_{q}")
                            pss.append(psq)
                        # weight-stationary: each lhsT feeds 4 consecutive
                        # matmuls
                        for ty in range(3):
                            for q in range(4):
                                lr = TILE_ROWS * (4 * th + q)
                                rp = _rows_ap(x2, (lr + ty) * WP, 0, 2 * C, co)
                                nc.tensor.matmul(
                                    pss[q], wp_t[ty], rp,
                                    start=(ty == 0), stop=False,
                                )
                        for q in range(4):
                            lr = TILE_ROWS * (4 * th + q)
                            rb = _rows_ap(xB, lr * WP + co, 0, 2 * C, 1)
                            nc.tensor.matmul(
                                pss[q], wm_t[0], rb,
                                start=False, stop=False,
                            )
                        for q in range(4):
                            lr = TILE_ROWS * (4 * th + q)
                            rs = _rows_ap(x2, (lr + 2) * WP + co, 0, 2 * C, 1)
                            nc.tensor.matmul(
                                pss[q], wm_t[1], rs,
                                start=False, stop=True,
                            )
                        # ACT for this block (runs while next block's convs
                        # occupy PE)
                        rts = []
                        for q in range(4):
                            t = 4 * th + q
                            r = rpool.tile([2 * C, NT], BF16, tag="r",
                                           name=f"r_{b}_{g}_{th}_{q}")
                            nc.scalar.activation(
                                out=r, in_=pss[q],
                                func=mybir.ActivationFunctionType.Relu,
                                bias=bias_t[:, b:b + 1], scale=1.0,
                            )
                            rts.append(r)
                        # previous block's fold+residual now that its ACT
                        # had a full block of conv time to finish
                        flush_pend()
                        obase = b * C * H * W + row0 * W + 4 * th * NT
                        dsto = AP(out.tensor, obase,
                                  [[H * W, C], [1, 4 * NT]])
                        pend.append((pss, rts, og, x2, co, th, dsto))
            flush_pend()
    nc.compile()
    return nc


def _get_nc():
    global _NC_CACHE
    if _NC_CACHE is None:
        _NC_CACHE = _build_nc()
    return _NC_CACHE


def _route(x, k, gate_w, gate_b):
    """Host gate: returns (idx [B,2] int, topw [B,2] f32), matching
    softmax(fp32) + lax.top_k (descending, ties -> lower index)."""
    pooled = x.mean(axis=(2, 3), dtype=np.float32)
    logits = (pooled @ gate_w.T + gate_b).astype(np.float32)
    m = logits.max(axis=1, keepdims=True)
    e = np.exp(logits - m)
    wts = (e / e.sum(axis=1, keepdims=True)).astype(np.float32)
    idx = np.argsort(-wts, axis=1, kind="stable")[:, :2]
    topw = np.take_along_axis(wts, idx, axis=1)
    return idx, topw


def _prep_inputs(inputs, k, gate_w, gate_b, conv_w, conv_b):
    x = np.asarray(inputs, np.float32)
    k = np.asarray(k, np.float32)
    gate_w = np.asarray(gate_w, np.float32)
    gate_b = np.asarray(gate_b, np.float32)
    conv_w = np.asarray(conv_w, np.float32)
    conv_b = np.asarray(conv_b, np.float32)

    idx, topw = _route(x, k, gate_w, gate_b)
    kk = k[:, :, 0, 0]                                  # [B, C]
    s = kk[:, None, :] * topw[:, :, None]               # [B, 2, C_out]
    wsel = conv_w[idx]                                  # [B, 2, co, ci, 3, 3]
    wsc = wsel * s[:, :, :, None, None, None]
    lhsT = np.transpose(wsc, (0, 4, 5, 3, 1, 2)).reshape(B, 3, 3, C, 2 * C)
    # pair slots: rows 0-63 = tap (ty,0), rows 64-127 = tap (ty,2)
    wpr = np.stack(
        [np.concatenate([lhsT[:, ty, 0], lhsT[:, ty, 2]], axis=1) for ty in range(3)],
        axis=1,
    )                                                   # [B, 3, 128, 128]
    wmid = np.zeros((B, 2, 2 * C, 2 * C), np.float32)
    wmid[:, 0, :C, :] = lhsT[:, 0, 1]      # pair slot: tap (0,1)
    wmid[:, 0, C:, :] = lhsT[:, 1, 1]      # pair slot: tap (1,1)
    wmid[:, 1, :C, :] = lhsT[:, 2, 1]      # single: tap (2,1), K padded
    bias_full = (conv_b[idx] * s).reshape(B, 2 * C)     # [B, 128]
    xpad = np.zeros((B, C, XPITCH), np.float32)
    pad2d = np.zeros((B, C, HP, WP), np.float32)
    pad2d[:, :, 1:H + 1, 1:W + 1] = x
    xpad[:, :, XS:XS + HP * WP] = pad2d.reshape(B, C, HP * WP)
    sel_in = np.ascontiguousarray(
        np.concatenate([np.eye(C, dtype=np.float32)] * 2, axis=0)
    )

    in_maps = []
    for c in range(N_CORES):
        sl = slice(c * BPC, (c + 1) * BPC)
        in_maps.append({
            "xpad": np.ascontiguousarray(xpad[sl]).astype(ml_dtypes.bfloat16),
            "wpr": np.ascontiguousarray(wpr[sl]).astype(ml_dtypes.bfloat16),
            "wmid": np.ascontiguousarray(wmid[sl]).astype(ml_dtypes.bfloat16),
            "bias": np.ascontiguousarray(bias_full[sl].T),
            "sel": sel_in.astype(ml_dtypes.bfloat16),
        })
    return in_maps


def _run(inputs, k, gate_w, gate_b, conv_w, conv_b, **run_kwargs):
    in_maps = _prep_inputs(inputs, k, gate_w, gate_b, conv_w, conv_b)
    nc = _get_nc()
    res = run_bass_kernel_spmd(nc, in_maps, core_ids=list(range(N_CORES)), **run_kwargs)
    outs = [
        np.asarray(res.results[c]["out"], dtype=np.float32).reshape(BPC, C, H, W)
        for c in range(N_CORES)
    ]
    full = np.concatenate(outs, axis=0)
    return full, res


def kernel(inputs, k, gate_w, gate_b, conv_w, conv_b):
    full, _ = _run(inputs, k, gate_w, gate_b, conv_w, conv_b)
    return full


# revision 3
# speedup vs baseline: 1.4866x; 1.2493x over previous
"""MoE conv layer (top-2 of 4 experts, 3x3 SAME conv + FiLM) on 8 TRN2 cores.

Strategy: data-parallel over batch (2 images per core); the tiny gate
(global-avg-pool -> linear -> softmax -> top-2) runs on the host and the
FiLM scale k and softmax weight are folded into the conv weights/bias
(valid since both are >= 0 and relu(z)*s == relu(z*s) for s >= 0).
bf16 datapath (psum f32), software-pipelined epilogue, staggered-reset
timing loop, split startup loads, q-major tail block.
Stores + const loads ride the ACT engine HWDGE queue (SP keeps x/w loads);
first window split into quarters so the first matmul starts ~2us in;
each 4-tile block stores its half-group as soon as its epilogue runs.

Data-parallel over batch (2 images per core). Host gate folds k*softmax
weight into conv weights (relu(z*s) == relu(z)*s for s >= 0).

Device kernel, per image, per 32-row group (all tensors bf16, psum f32):
  Two half-window DMAs load x2 [128, XW]: 64 channels of the zero-padded
  window (lo) and the same window shifted +2 cols (hi) — two DMAs, not one
  fused, and K=128 everywhere: measured faster on HW than the "obvious"
  single-DMA / K=64 variants.
  conv per 512-pixel (4-row) tile: 6 matmuls accumulating in PSUM
    - 3 "pair" matmuls (K=128): taps (ty,0)+(ty,2) via the shifted stacking
    - 3 "single" matmuls (K=128, hi half zero weights): taps (ty,1)
    both experts packed in M=128 (e0 -> psum 0-63, e1 -> 64-127)
  epilogue per tile (pipelined one 4-tile block behind the convs so PE
  never waits on ACT):
    ACT: r = relu(psum + bias)            (bias = conv_b * k * w)
    PE : psum[0:64] = [I;I].T @ r         (expert fold, written back into
                                           the SAME psum tile - no extra
                                           PSUM banks, psA uses all 8)
    DVE: og[tile] = psum[0:64] + x        (residual, lane-aligned)
  one store DMA per group (bf16; host casts back to f32).
"""

import contextlib

import numpy as np
import ml_dtypes

import concourse.bass as bass
import concourse.tile as tile
from concourse import bacc, mybir
from concourse.ap import AP
from concourse.bass_utils import run_bass_kernel_spmd

N_CORES = 8
B, C, H, W = 16, 64, 128, 128
BPC = B // N_CORES            # images per core
HP, WP = H + 2, W + 2         # zero-padded image dims (SAME conv)
GROUP_ROWS = 32               # output rows per x-window group
N_GROUPS = H // GROUP_ROWS    # 4
TILE_ROWS = 4                 # output rows per matmul tile (N = 4*128 = 512)
TILES_PER_GROUP = GROUP_ROWS // TILE_ROWS   # 8
NT = TILE_ROWS * W            # matmul free size (512)
WIN = (GROUP_ROWS + 2) * WP   # x-window cols per group (incl. 2 halo rows)
XW = WIN + 2                  # x2 tile width (+2 so odd-group reads stay in range)
GPIX = GROUP_ROWS * W         # output pixels per group (4096)
XPITCH = HP * WP + 2 + WP + 4  # per-channel pitch (2 lead, WP+4 trail for xB hi)
XS = 2                        # offset of the real window start within a channel

F32 = mybir.dt.float32
BF16 = mybir.dt.bfloat16

_NC_CACHE = None


def _rows_ap(t_ap, base_col, p0, p1, col0, ncol=W):
    """TILE_ROWS rows of stride WP starting at base_col within an x2 tile."""
    v = t_ap[p0:p1, base_col:base_col + TILE_ROWS * WP]
    v = v.rearrange("p (r c) -> p r c", c=WP)
    return v[:, :, col0:col0 + ncol]


def _build_nc(loop_n=None):
    nc = bacc.Bacc("TRN2", target_bir_lowering=False, debug=False)
    xpad = nc.dram_tensor("xpad", [BPC, C, XPITCH], BF16, kind="ExternalInput").ap()
    wpr = nc.dram_tensor("wpr", [BPC, 3, 2 * C, 2 * C], BF16, kind="ExternalInput").ap()
    wmid = nc.dram_tensor("wmid", [BPC, 2, 2 * C, 2 * C], BF16, kind="ExternalInput").ap()
    bias = nc.dram_tensor("bias", [2 * C, BPC], F32, kind="ExternalInput").ap()
    sel = nc.dram_tensor("sel", [2 * C, C], BF16, kind="ExternalInput").ap()
    out = nc.dram_tensor("out", [BPC, C, H * W], BF16, kind="ExternalOutput").ap()

    with tile.TileContext(nc) as tc:
        loop_ctx = (
            tc.For_i(0, loop_n, 1, staggered_reset=True, hint_engines=(
                mybir.EngineType.PE, mybir.EngineType.Activation,
                mybir.EngineType.DVE, mybir.EngineType.SP,
            ))
            if loop_n is not None else contextlib.nullcontext()
        )
        with (
            loop_ctx,
            tc.tile_pool(name="const", bufs=1) as cpool,
            tc.tile_pool(name="wpool", bufs=6) as wpool,
            tc.tile_pool(name="xpool", bufs=6) as xpool,
            tc.tile_pool(name="rpool", bufs=6) as rpool,
            tc.tile_pool(name="opool", bufs=4) as opool,
            tc.tile_pool(name="psA", bufs=8, space="PSUM") as psA,
        ):
            # first window of the first image before anything else, split in
            # row-halves: the first 4-tile block only needs rows < 18, so the
            # first matmul can start after the first quarter lands
            x2_first = xpool.tile([2 * C, XW], BF16, tag="x2")
            base0 = 0 * C * XPITCH + XS
            XSPL = 18 * WP
            for (p0, p1, off) in ((0, C, 0), (C, 2 * C, 2)):
                nc.sync.dma_start(
                    out=x2_first[p0:p1, 0:XSPL],
                    in_=AP(xpad.tensor, base0 + off, [[XPITCH, C], [1, XSPL]]))
            for (p0, p1, off) in ((0, C, 0), (C, 2 * C, 2)):
                nc.sync.dma_start(
                    out=x2_first[p0:p1, XSPL:XW],
                    in_=AP(xpad.tensor, base0 + off + XSPL,
                           [[XPITCH, C], [1, XW - XSPL]]))
            xB_first = xpool.tile([2 * C, XW], BF16, tag="xb", bufs=4)
            for (lo, hi) in ((0, XSPL), (XSPL, XW)):
                nc.sync.dma_start(
                    out=xB_first[0:C, lo:hi],
                    in_=AP(xpad.tensor, base0 + lo, [[XPITCH, C], [1, hi - lo]]))
                nc.sync.dma_start(
                    out=xB_first[C:2 * C, lo:hi],
                    in_=AP(xpad.tensor, base0 + WP + lo,
                           [[XPITCH, C], [1, hi - lo]]))
            # consts off the SP queue: ACT hosts its own HWDGE
            sel_t = cpool.tile([2 * C, C], BF16)
            nc.scalar.dma_start(out=sel_t, in_=sel)
            bias_t = cpool.tile([2 * C, BPC], F32)
            nc.scalar.dma_start(out=bias_t, in_=bias)

            # pending epilogue from the previous 4-tile block:
            # (pss, rts, og, x2, co, th, store) -- emitted AFTER the next
            # block's conv matmuls so PE never waits on ACT; the group store
            # rides along with the group's last block so it lands after the
            # DVE writes it needs
            pend = []

            def flush_pend():
                for (pss, rts, og, x2v, co, th, store) in pend:
                    for q in range(4):
                        t = 4 * th + q
                        lr = TILE_ROWS * t
                        ps2 = pss[q][0:C, :]
                        nc.tensor.matmul(ps2, sel_t, rts[q], start=True,
                                         stop=True)
                        xres = _rows_ap(x2v, (lr + 1) * WP + co, 0, C, 1)
                        nc.vector.tensor_add(
                            out=og[:, NT * t:NT * (t + 1)],
                            in0=ps2, in1=xres,
                        )
                    if store is not None:
                        nc.scalar.dma_start(out=store, in_=og[:, NT * 4 * th:
                                                              NT * 4 * (th + 1)])
                pend.clear()

            for b in range(BPC):
                wp_t, wm_t = [], []
                for ty in range(3):
                    wt = wpool.tile([2 * C, 2 * C], BF16, tag="wp")
                    nc.sync.dma_start(out=wt, in_=wpr[b, ty])
                    wp_t.append(wt)
                for mi in range(2):
                    # wm[0]: K lo=tap(0,1), hi=tap(1,1) pair via xB
                    # wm[1]: K lo=tap(2,1), hi=zeros (K<128 measures slower
                    # on HW, so pad K)
                    st = wpool.tile([2 * C, 2 * C], BF16, tag="ws")
                    nc.sync.dma_start(out=st, in_=wmid[b, mi])
                    wm_t.append(st)
                for g in range(N_GROUPS):
                    j = g % 2
                    row0 = GROUP_ROWS * g
                    if b == 0 and g == 0:
                        x2 = x2_first
                    else:
                        x2 = xpool.tile([2 * C, XW], BF16, tag="x2")
                        # even groups: lo = x, hi = x shifted +2
                        # odd groups:  lo = x shifted -2, hi = x
                        base = b * C * XPITCH + XS + WP * row0 - 2 * j
                        src_lo = AP(xpad.tensor, base, [[XPITCH, C], [1, XW]])
                        src_hi = AP(xpad.tensor, base + 2, [[XPITCH, C], [1, XW]])
                        nc.sync.dma_start(out=x2[0:C, :], in_=src_lo)
                        nc.sync.dma_start(out=x2[C:2 * C, :], in_=src_hi)
                    if b == 0 and g == 0:
                        xB = xB_first
                    else:
                        xB = xpool.tile([2 * C, XW], BF16, tag="xb", bufs=4)
                        # lo = x (shifted -2j), hi = x shifted +WP (row down)
                        baseB = b * C * XPITCH + XS + WP * row0 - 2 * j
                        nc.sync.dma_start(
                            out=xB[0:C, :],
                            in_=AP(xpad.tensor, baseB, [[XPITCH, C], [1, XW]]))
                        nc.sync.dma_start(
                            out=xB[C:2 * C, :],
                            in_=AP(xpad.tensor, baseB + WP,
                                   [[XPITCH, C], [1, XW]]))
                    # epilogue reads use the UNSHIFTED copy: shift cols by +2j
                    co = 2 * j
                    og = opool.tile([C, GPIX], BF16, tag="og")
                    for th in range(TILES_PER_GROUP // 4):
                        final_blk = (b == BPC - 1 and g == N_GROUPS - 1
                                     and th == TILES_PER_GROUP // 4 - 1)
                        pss = []
                        for q in range(4):
                            psq = psA.tile([2 * C, NT], F32, tag="psA",
                                           name=f"ps_{b}_{g}_{th}_{q}")
                            pss.append(psq)
                        if final_blk:
                            # q-major: psum q completes after 5 matmuls so
                            # its ACT overlaps the rest of the tail
                            for q in range(4):
                                lr = TILE_ROWS * (4 * th + q)
                                for ty in range(3):
                                    rp = _rows_ap(x2, (lr + ty) * WP, 0,
                                                  2 * C, co)
                                    nc.tensor.matmul(
                                        pss[q], wp_t[ty], rp,
                                        start=(ty == 0), stop=False)
                                rb = _rows_ap(xB, lr * WP + co, 0, 2 * C, 1)
                                nc.tensor.matmul(pss[q], wm_t[0], rb,
                                                 start=False, stop=False)
                                rs = _rows_ap(x2, (lr + 2) * WP + co, 0,
                                              2 * C, 1)
                                nc.tensor.matmul(pss[q], wm_t[1], rs,
                                                 start=False, stop=True)
                        else:
                            for ty in range(3):
                                for q in range(4):
                                    lr = TILE_ROWS * (4 * th + q)
                                    rp = _rows_ap(x2, (lr + ty) * WP, 0,
                                                  2 * C, co)
                                    nc.tensor.matmul(
                                        pss[q], wp_t[ty], rp,
                                        start=(ty == 0), stop=False,
                                    )
                            for q in range(4):
                                lr = TILE_ROWS * (4 * th + q)
                                rb = _rows_ap(xB, lr * WP + co, 0, 2 * C, 1)
                                nc.tensor.matmul(
                                    pss[q], wm_t[0], rb,
                                    start=False, stop=False,
                                )
                            for q in range(4):
                                lr = TILE_ROWS * (4 * th + q)
                                rs = _rows_ap(x2, (lr + 2) * WP + co, 0,
                                              2 * C, 1)
                                nc.tensor.matmul(
                                    pss[q], wm_t[1], rs,
                                    start=False, stop=True,
                                )
                        # ACT for this block (runs while next block's convs
                        # occupy PE)
                        rts = []
                        for q in range(4):
                            t = 4 * th + q
                            r = rpool.tile([2 * C, NT], BF16, tag="r",
                                           name=f"r_{b}_{g}_{th}_{q}")
                            nc.scalar.activation(
                                out=r, in_=pss[q],
                                func=mybir.ActivationFunctionType.Relu,
                                bias=bias_t[:, b:b + 1], scale=1.0,
                            )
                            rts.append(r)
                        # previous block's fold+residual now that its ACT
                        # had a full block of conv time to finish
                        flush_pend()
                        obase = b * C * H * W + row0 * W + 4 * th * NT
                        dsto = AP(out.tensor, obase,
                                  [[H * W, C], [1, 4 * NT]])
                        pend.append((pss, rts, og, x2, co, th, dsto))
            flush_pend()
    nc.compile()
    return nc


def _get_nc():
    global _NC_CACHE
    if _NC_CACHE is None:
        _NC_CACHE = _build_nc()
    return _NC_CACHE


def _route(x, k, gate_w, gate_b):
    """Host gate: returns (idx [B,2] int, topw [B,2] f32), matching
    softmax(fp32) + lax.top_k (descending, ties -> lower index)."""
    pooled = x.mean(axis=(2, 3), dtype=np.float32)
    logits = (pooled @ gate_w.T + gate_b).astype(np.float32)
    m = logits.max(axis=1, keepdims=True)
    e = np.exp(logits - m)
    wts = (e / e.sum(axis=1, keepdims=True)).astype(np.float32)
    idx = np.argsort(-wts, axis=1, kind="stable")[:, :2]
    topw = np.take_along_axis(wts, idx, axis=1)
    return idx, topw


def _prep_inputs(inputs, k, gate_w, gate_b, conv_w, conv_b):
    x = np.asarray(inputs, np.float32)
    k = np.asarray(k, np.float32)
    gate_w = np.asarray(gate_w, np.float32)
    gate_b = np.asarray(gate_b, np.float32)
    conv_w = np.asarray(conv_w, np.float32)
    conv_b = np.asarray(conv_b, np.float32)

    idx, topw = _route(x, k, gate_w, gate_b)
    kk = k[:, :, 0, 0]                                  # [B, C]
    s = kk[:, None, :] * topw[:, :, None]               # [B, 2, C_out]
    wsel = conv_w[idx]                                  # [B, 2, co, ci, 3, 3]
    wsc = wsel * s[:, :, :, None, None, None]
    lhsT = np.transpose(wsc, (0, 4, 5, 3, 1, 2)).reshape(B, 3, 3, C, 2 * C)
    # pair slots: rows 0-63 = tap (ty,0), rows 64-127 = tap (ty,2)
    wpr = np.stack(
        [np.concatenate([lhsT[:, ty, 0], lhsT[:, ty, 2]], axis=1) for ty in range(3)],
        axis=1,
    )                                                   # [B, 3, 128, 128]
    wmid = np.zeros((B, 2, 2 * C, 2 * C), np.float32)
    wmid[:, 0, :C, :] = lhsT[:, 0, 1]      # pair slot: tap (0,1)
    wmid[:, 0, C:, :] = lhsT[:, 1, 1]      # pair slot: tap (1,1)
    wmid[:, 1, :C, :] = lhsT[:, 2, 1]      # single: tap (2,1), K padded
    bias_full = (conv_b[idx] * s).reshape(B, 2 * C)     # [B, 128]
    xpad = np.zeros((B, C, XPITCH), np.float32)
    pad2d = np.zeros((B, C, HP, WP), np.float32)
    pad2d[:, :, 1:H + 1, 1:W + 1] = x
    xpad[:, :, XS:XS + HP * WP] = pad2d.reshape(B, C, HP * WP)
    sel_in = np.ascontiguousarray(
        np.concatenate([np.eye(C, dtype=np.float32)] * 2, axis=0)
    )

    in_maps = []
    for c in range(N_CORES):
        sl = slice(c * BPC, (c + 1) * BPC)
        in_maps.append({
            "xpad": np.ascontiguousarray(xpad[sl]).astype(ml_dtypes.bfloat16),
            "wpr": np.ascontiguousarray(wpr[sl]).astype(ml_dtypes.bfloat16),
            "wmid": np.ascontiguousarray(wmid[sl]).astype(ml_dtypes.bfloat16),
            "bias": np.ascontiguousarray(bias_full[sl].T),
            "sel": sel_in.astype(ml_dtypes.bfloat16),
        })
    return in_maps


def _run(inputs, k, gate_w, gate_b, conv_w, conv_b, **run_kwargs):
    in_maps = _prep_inputs(inputs, k, gate_w, gate_b, conv_w, conv_b)
    nc = _get_nc()
    res = run_bass_kernel_spmd(nc, in_maps, core_ids=list(range(N_CORES)), **run_kwargs)
    outs = [
        np.asarray(res.results[c]["out"], dtype=np.float32).reshape(BPC, C, H, W)
        for c in range(N_CORES)
    ]
    full = np.concatenate(outs, axis=0)
    return full, res


def kernel(inputs, k, gate_w, gate_b, conv_w, conv_b):
    full, _ = _run(inputs, k, gate_w, gate_b, conv_w, conv_b)
    return full
